# revision 57
# baseline (speedup 1.0000x reference)
"""DGCNN forward (2x dynamic-kNN EdgeConv + classifier) on 8 Trainium2 cores.

Data-parallel over the B=8 point clouds: core b handles cloud b (4096 points),
fully fused on-chip:

  kNN   - augmented f16 matmul gives negdist = 2*x_i.x_j - |x_j|^2 - |x_i|^2
          straight in PSUM (1 PE cycle/row vs 4 for f32); the self column is
          pre-killed with a predicated diagonal write; one DVE pass packs the
          column index j into the low 12 mantissa bits (negdist quantized to
          ~2^-11 relative); a sub-chunk(8)-max reduce gives 512 packed
          sub-chunk maxima per point and three max8/match_replace rounds
          select the top-24; the top-20 sub-chunk maxima ARE the neighbor
          list (approximate kNN: a true neighbor is only replaced when two of
          the top-20 share an 8-wide index sub-chunk, which the EdgeConv max
          aggregation washes out). Everything stays in SBUF - no DRAM
          round-trip, no indirect DMA.
  EConv - neighbor features move SBUF->SBUF with ap_gather driven by a
          wrapped index list built with two PE transposes; the MLP runs
          feature-major in f16 (stationary weights, 1 cycle/row); max over
          the 20 neighbors is fused into the PSUM evacuation of the last
          layer.
  Head  - lin0 feature-major f16; segment-max uses host-prepared index lists
          (from the sorted batch labels) gathered against -inf-augmented
          tables; a 16KB AllReduce-max merges the per-core [8,512] partials;
          each core runs the tiny classifier + log_softmax in f32.
"""

import contextlib

import numpy as np

import bass_rust
import concourse.bass as bass
import concourse.bacc as bacc
import concourse.mybir as mybir
from concourse import masks
from concourse.bass import _add_dep_helper
from concourse.tile import TileContext
from concourse.vector_clock import ScopedClock

dt = mybir.dt
Alu = mybir.AluOpType
Act = mybir.ActivationFunctionType

B, N, KNN, NCLS = 8, 4096, 20, 40
P = 128
NT = N // P            # 32 point tiles per core
CH = 8                 # sub-chunk size for the maxima hierarchy
NCH = N // CH          # 512 sub-chunks per row
NC3 = 24               # sub-chunk maxima kept (3 max8 rounds)
KPAD = 32              # padded K for the wrapped gather list
NEG = -1.0e30
MASK_HI = -4096        # 0xFFFFF000 as signed int32
SEG_PTS = 256          # boundary-point slots per segment
Q = 1024               # dist quarter width
NQ = N // Q


# --------------------------------------------------------------------------
# This walrus build rejects Drain instructions carrying >1 sync wait; split
# the TileContext tail-drain waits across single-wait nops.
def _patched_drain_and_barrier(self, tick_clock, wait_clock):
    nc = self.nc
    probe = nc.sync.nop(nofuse=True)
    wait_clock.add_sem_waits(probe.ins, ScopedClock({None: tick_clock.global_clock}))
    si = probe.ins.sync_info
    waits = list(si.on_wait) if si is not None else []
    if len(waits) > 1:
        probe.ins.sync_info = bass_rust.SyncInfo(
            on_wait=[waits[0]], on_update=list(si.on_update)
        )
        for w in waits[1:]:
            extra = nc.sync.nop(nofuse=True)
            extra.ins.sync_info = bass_rust.SyncInfo(on_wait=[w], on_update=[])
    nc.sync.drain()
    nc.all_engine_barrier()
    assert self.sems is not None
    popped = nc._tile_sem_poison_stack.pop()
    assert popped is self._sem_poison
    nc.clear_and_free_semaphores(list(self.sems.allocated().values()))
    nc.all_engine_barrier()


TileContext._drain_and_barrier = _patched_drain_and_barrier


def _wrap16(lst, cols):
    """[i % 16, i // 16] wrapped layout used by the gpsimd gather ops."""
    a = np.asarray(lst, dtype=np.int16)
    assert a.size == 16 * cols, (a.size, cols)
    return a.reshape(cols, 16).T.copy()


# --------------------------------------------------------------------------
def _knn_layer(nc, pools, KA, x_aug, x2r, st, dump_d=None):
    """One kNN layer (fully in SBUF); returns per-tile wrapped gather lists."""
    sb, psum, smalls = pools["sb"], pools["psum"], pools["smalls"]
    widx_tiles = []
    for t in range(NT):
        lhsT = x_aug[0:KA, t * P : (t + 1) * P]
        cmax = smalls.tile([P, NCH], dt.float32, tag="cmax")
        for q in range(NQ):
            pq = psum.tile([P, Q], dt.float32, tag="dist")
            for h in range(Q // 512):
                j0 = q * Q + h * 512
                nc.tensor.matmul(
                    pq[:, h * 512 : (h + 1) * 512],
                    lhsT,
                    x2r[0:KA, j0 : j0 + 512],
                    start=True,
                    stop=True,
                )
            if q == (t * P) // Q:
                # negdist(i,i) == 0 is always the row max; kill it in PSUM
                off = (t * P) % Q
                nc.vector.copy_predicated(
                    pq[:, off : off + P],
                    st["identity"][:].bitcast(dt.uint32),
                    st["negbig"][:],
                )
            packed = sb.tile([P, Q], dt.float32, tag="packed")
            # packed = (negdist & 0xFFFFF000) | j
            sttp = nc.vector.scalar_tensor_tensor(
                out=packed[:].bitcast(dt.int32),
                in0=pq[:].bitcast(dt.int32),
                scalar=MASK_HI,
                in1=st["iota_j"][:, q * Q : (q + 1) * Q],
                op0=Alu.bitwise_and,
                op1=Alu.bitwise_or,
            )
            # walrus wants an integer ImmVal for bitvec ops
            _il = sttp.ins.ins
            _il[1] = mybir.ImmediateValue(dtype=dt.int32, value=MASK_HI)
            sttp.ins.ins = _il
            nc.vector.tensor_reduce(
                out=cmax[:, q * (Q // CH) : (q + 1) * (Q // CH)],
                in_=packed[:].rearrange("p (c s) -> p c s", s=CH),
                axis=mybir.AxisListType.X,
                op=Alu.max,
            )
        # ---- top-24 sub-chunk maxima; ids live in the low 12 bits ----
        if t == 0 and dump_d is not None:
            nc.sync.dma_start(out=dump_d["dbg_cmax"][:], in_=cmax[:])
        m8 = smalls.tile([P, NC3], dt.float32, tag="m8")
        cwork = smalls.tile([P, NCH], dt.float32, tag="cwork")
        src = cmax
        for r in range(3):
            nc.vector.max(out=m8[:, r * 8 : (r + 1) * 8], in_=src[:])
            if r < 2:
                nc.vector.match_replace(
                    out=cwork[:],
                    in_to_replace=m8[:, r * 8 : (r + 1) * 8],
                    in_values=src[:],
                    imm_value=NEG,
                )
                src = cwork
        if t == 0 and dump_d is not None:
            nc.sync.dma_start(out=dump_d["dbg_m8"][:], in_=m8[:])
        gidx = smalls.tile([P, KNN], dt.int32, tag="gidx")
        nc.vector.tensor_scalar(
            out=gidx[:], in0=m8[:, 0:KNN].bitcast(dt.int32), scalar1=0xFFF,
            scalar2=None, op0=Alu.bitwise_and,
        )
        # ---- wrapped gather list via two PE transposes ----
        gf = smalls.tile([P, KPAD], dt.float32, tag="gf")
        nc.vector.tensor_copy(gf[:, 0:KNN], gidx[:])
        nc.vector.tensor_copy(
            gf[:, KNN:KPAD], gidx[:, 0:1].to_broadcast([P, KPAD - KNN])
        )
        w32 = smalls.tile([16, 2 * P], dt.float32, tag="w32")
        w32v = w32[:].rearrange("r (c two) -> r c two", two=2)
        for half in range(2):
            tp = psum.tile([16, P], dt.float32, tag="tp")
            nc.tensor.transpose(
                tp[:], gf[:, half * 16 : (half + 1) * 16], st["identity"][:]
            )
            nc.vector.tensor_copy(w32v[:, :, half], tp[:])
        widx = smalls.tile([16, 2 * P], dt.int16, tag="widx")
        nc.vector.tensor_copy(widx[:], w32[:])
        widx_tiles.append(widx)
    return widx_tiles


# --------------------------------------------------------------------------
def _edgeconv(nc, pools, D, KA, DMID, DOUT, x_aug, x_gsrc, widx_tiles, wmm1,
              wmm2, w2t, b2t, w3t, b3t, x_out, gtab_rows):
    """One EdgeConv; pooled relu output written to x_out (feature-major,
    [P, (DOUT//P or 1)*N] layout, block b at columns [b*N, (b+1)*N)).

    x_aug is the f16 feature tensor (rhs_i / matmul side); x_gsrc is an f32
    copy of the feature rows used as the ap_gather source (the gpsimd gather
    needs 4-byte elements); gathered rows are converted to f16 per tile."""
    sb, psum, smalls = pools["sb"], pools["psum"], pools["smalls"]
    chunks = [(0, 25), (25, 25), (50, 25), (75, 25), (100, 25), (125, 3)]
    NB3 = max(1, DOUT // P)
    for t in range(NT):
        widx = widx_tiles[t]
        gath = sb.tile([gtab_rows, P * KPAD], dt.float32, tag="gath", bufs=1)
        if gtab_rows > 16:
            wrep = smalls.tile([gtab_rows, 2 * P], dt.int16, tag="wrep")
            for g in range(gtab_rows // 16):
                nc.sync.dma_start(out=wrep[g * 16 : (g + 1) * 16, :], in_=widx[:])
            idxs = wrep
        else:
            idxs = widx
        nc.gpsimd.ap_gather(
            out_ap=gath[:].rearrange("c (i one) -> c i one", one=1),
            in_ap=x_gsrc[0:gtab_rows, :].rearrange("c (e one) -> c e one", one=1),
            idxs_ap=idxs[:],
            channels=gtab_rows,
            num_elems=N,
            d=1,
            num_idxs=P * KPAD,
        )
        gath16 = sb.tile([D, P * KPAD], dt.float16, tag="gath16")
        nc.vector.tensor_copy(gath16[:], gath[0:D, :])
        gview = gath16[:].rearrange("c (p k) -> c p k", k=KPAD)
        for (p0, pn) in chunks:
            ns = pn * KNN
            # h1 = relu(x_i @ (W1a-W1b) + b1 + x_j @ W1b)
            ph1 = psum.tile([DMID, 512], dt.float32, tag="mlp")
            rhs_i = (
                x_aug[0:KA, t * P + p0 : t * P + p0 + pn]
                .rearrange("c (p one) -> c p one", one=1)
                .to_broadcast([KA, pn, KNN])
            )
            nc.tensor.matmul(ph1[:, 0:ns], wmm1[:], rhs_i, start=True, stop=False)
            nc.tensor.matmul(
                ph1[:, 0:ns], wmm2[:], gview[0:D, p0 : p0 + pn, 0:KNN],
                start=False, stop=True,
            )
            h1 = sb.tile([DMID, 512], dt.float16, tag="h1")
            nc.scalar.activation(h1[:, 0:ns], ph1[:, 0:ns], Act.Relu)
            # h2 = relu(h1 @ W2 + b2)
            ph2 = psum.tile([DMID, 512], dt.float32, tag="mlp")
            nc.tensor.matmul(ph2[:, 0:ns], w2t[:], h1[:, 0:ns], start=True, stop=True)
            h2 = sb.tile([DMID, 512], dt.float16, tag="h2")
            nc.scalar.activation(
                h2[:, 0:ns], ph2[:, 0:ns], Act.Relu, bias=b2t[:, 0:1]
            )
            # h3 = h2 @ W3 ; max over k ; relu(. + b3)
            for b3 in range(NB3):
                mw = min(P, DOUT)
                ph3 = psum.tile([P, 512], dt.float32, tag="mlp")
                nc.tensor.matmul(
                    ph3[0:mw, 0:ns], w3t[:, b3 * P : b3 * P + mw], h2[:, 0:ns],
                    start=True, stop=True,
                )
                pooled = smalls.tile([P, 32], dt.float32, tag="pooled")
                nc.vector.tensor_reduce(
                    out=pooled[0:mw, 0:pn],
                    in_=ph3[0:mw, 0:ns].rearrange("c (p k) -> c p k", k=KNN),
                    axis=mybir.AxisListType.X,
                    op=Alu.max,
                )
                nc.scalar.activation(
                    x_out[0:mw, b3 * N + t * P + p0 : b3 * N + t * P + p0 + pn],
                    pooled[0:mw, 0:pn],
                    Act.Relu,
                    bias=b3t[0:mw, b3 : b3 + 1],
                )


# --------------------------------------------------------------------------
def build(collective=True, debug=False, dumps=False):
    nc = bacc.Bacc(
        "TRN2", target_bir_lowering=False, debug=debug,
        num_devices=B if collective else 1,
    )
    f32 = dt.float32
    dump_d = {}
    if dumps:
        for nm, shp, dtp in [
            ("dbg_x0", [65, 16], f32), ("dbg_widx", [16, 256], dt.int16),
            ("dbg_x1g", [64, 16], f32), ("dbg_pmax", [P, 4 * B], f32),
            ("dbg_smax", [P, 4 * B], f32), ("dbg_x2f", [P, 16], f32),
            ("dbg_iota", [1, 64], dt.int32),
            ("dbg_cmax", [P, NCH], f32), ("dbg_m8", [P, NC3], f32),
            ("dbg_hfm", [P, 16], f32),
            ("dbg_tm", [P, 33], f32), ("dbg_x1_full", [64, N], f32),
            ("dbg_x2_full", [P, 2 * N], dt.float16),
            ("dbg_hfm0", [P, N], f32),
        ]:
            dump_d[nm] = nc.dram_tensor(nm, shp, dtp, kind="ExternalOutput")

    def din(name, shape, dtype=f32):
        return nc.dram_tensor(name, shape, dtype, kind="ExternalInput")

    x0aug_d = din("x0aug", [65, N])
    psel_d = din("psel", [16, 16], dt.int16)
    hsel_d = din("hsel", [16, B * SEG_PTS // 16], dt.int16)
    w_m1 = din("m1w1", [6, 64]); b_m1 = din("m1b1", [64])
    w_m12 = din("m1w2", [64, 64]); b_m12 = din("m1b2", [64])
    w_m13 = din("m1w3", [64, 64]); b_m13 = din("m1b3", [64])
    w_m2 = din("m2w1", [P, P]); b_m2 = din("m2b1", [P])
    w_m22 = din("m2w2", [P, P]); b_m22 = din("m2b2", [P])
    w_m23 = din("m2w3", [P, 256]); b_m23 = din("m2b3", [256])
    lin0_w = din("lin0_w", [256, 512]); lin0_b = din("lin0_b", [512])
    lin1_w = din("lin1_w", [512, 256]); lin1_b = din("lin1_b", [256])
    lin2_w = din("lin2_w", [256, 256]); lin2_b = din("lin2_b", [256])
    lin3_w = din("lin3_w", [256, NCLS]); lin3_b = din("lin3_b", [NCLS])
    out_d = nc.dram_tensor("out", [B, NCLS], f32, kind="ExternalOutput")

    cc_in = nc.dram_tensor("cc_in", [P, 4 * B], f32)
    cc_out = nc.dram_tensor("cc_out", [P, 4 * B], f32, addr_space="Shared")

    with TileContext(nc) as tc, contextlib.ExitStack() as ctx:
        const = ctx.enter_context(tc.tile_pool(name="const", bufs=1))
        sb = ctx.enter_context(tc.tile_pool(name="sb", bufs=2))
        smalls = ctx.enter_context(tc.tile_pool(name="smalls", bufs=2))
        psum = ctx.enter_context(tc.tile_pool(name="psum", bufs=2, space="PSUM"))
        pools = {"sb": sb, "psum": psum, "smalls": smalls}

        f16 = dt.float16

        # ---- statics ----
        identity = const.tile([P, P], f32)
        masks.make_identity(nc, identity[:])
        iota_j = const.tile([P, N], dt.int32)
        nc.gpsimd.iota(iota_j[:], pattern=[[1, N]], base=0, channel_multiplier=0)
        if dumps:
            nc.sync.dma_start(out=dump_d["dbg_iota"][:], in_=iota_j[0:1, 0:64])
        negbig = const.tile([P, P], f32)
        nc.vector.memset(negbig[:], NEG)
        st = {"identity": identity, "iota_j": iota_j, "negbig": negbig}

        # ---- inputs / weights ----
        x0aug = const.tile([65, N], f32)
        nc.sync.dma_start(out=x0aug[:], in_=x0aug_d[:])
        x0aug16 = sb.tile([97, N], f16, tag="xaug16", bufs=1, name="x0aug16")
        nc.vector.tensor_copy(x0aug16[0:65, :], x0aug[:])
        if dumps:
            nc.sync.dma_start(out=dump_d["dbg_x0"][:], in_=x0aug[:, 0:16])

        _ldn = [0]

        def load(dr_ap, shape, pool=const, tag=None, bufs=None):
            if tag is None:
                _ldn[0] += 1
                tag = f"ld{_ldn[0]}"
            t_ = pool.tile(shape, f32, tag=tag, name=tag, bufs=bufs)
            nc.sync.dma_start(out=t_[:], in_=dr_ap)
            return t_

        def to16(src, shape):
            _ldn[0] += 1
            t_ = const.tile(shape, f16, tag=f"w16_{_ldn[0]}")
            nc.vector.tensor_copy(t_[:], src)
            return t_

        w1a = load(w_m1[0:3, :], [3, 64])
        w1b = load(w_m1[3:6, :], [3, 64])
        ec1_mm1 = const.tile([33, 64], f32)
        nc.vector.memset(ec1_mm1[:], 0.0)
        nc.vector.tensor_sub(ec1_mm1[0:3, :], w1a[:], w1b[:])
        nc.sync.dma_start(
            out=ec1_mm1[32:33, :], in_=b_m1[:].rearrange("(o x) -> o x", o=1)
        )
        ec1_mm1_h = to16(ec1_mm1[:], [33, 64])
        w1b_h = to16(w1b[:], [3, 64])
        ec1_w2 = to16(load(w_m12[:], [64, 64])[:], [64, 64])
        ec1_b2 = load(b_m12[:].rearrange("(x o) -> x o", o=1), [64, 1])
        ec1_w3 = to16(load(w_m13[:], [64, 64])[:], [64, 64])
        ec1_b3 = load(b_m13[:].rearrange("(x o) -> x o", o=1), [64, 1])

        w2a = load(w_m2[0:64, :], [64, P])
        w2b = load(w_m2[64:128, :], [64, P])
        ec2_mm1 = const.tile([65, P], f32)
        nc.vector.tensor_sub(ec2_mm1[0:64, :], w2a[:], w2b[:])
        nc.sync.dma_start(
            out=ec2_mm1[64:65, :], in_=b_m2[:].rearrange("(o x) -> o x", o=1)
        )
        ec2_mm1_h = to16(ec2_mm1[:], [65, P])
        w2b_h = to16(w2b[:], [64, P])
        ec2_w2 = to16(load(w_m22[:], [P, P])[:], [P, P])
        ec2_b2 = load(b_m22[:].rearrange("(x o) -> x o", o=1), [P, 1])
        ec2_w3 = to16(load(w_m23[:], [P, 256])[:], [P, 256])
        ec2_b3 = load(b_m23[:].rearrange("(o x) -> x o", o=2), [P, 2])

        # ---- x2r (dist rhs) builder ----
        def build_x2r(x_aug_t, D, lane1, lane2, tag):
            # negdist = -dist^2: x_aug has ones @ lane1, -|x|^2 @ lane2;
            # x2r has [2x ; -|x|^2 @ lane1 ; ones @ lane2]. All lane starts
            # are 32-aligned. KA = lane2 + 1.
            KA = lane2 + 1
            x2r = sb.tile([KA, N], f16, tag="x2r", bufs=1, name=tag)
            nc.vector.memset(x2r[:], 0.0)
            nc.vector.memset(x2r[lane2 : lane2 + 1, :], 1.0)
            nc.vector.tensor_scalar_mul(x2r[0:D, :], x_aug_t[0:D, :], 2.0)
            xsq = sb.tile([D, N], f16, tag="xsq", bufs=1)
            nc.vector.tensor_mul(xsq[:], x_aug_t[0:D, :], x_aug_t[0:D, :])
            ones_l = const.tile([D, 1], f16, tag=tag + "_ones")
            nc.vector.memset(ones_l[:], 1.0)
            for c in range(N // 512):
                pq = psum.tile([1, 512], f32, tag="tp")
                nc.tensor.matmul(
                    pq[:], ones_l[:], xsq[:, c * 512 : (c + 1) * 512],
                    start=True, stop=True,
                )
                nc.scalar.activation(
                    x2r[lane1 : lane1 + 1, c * 512 : (c + 1) * 512], pq[:],
                    Act.Copy, scale=-1.0,
                )
                nc.scalar.activation(
                    x_aug_t[lane2 : lane2 + 1, c * 512 : (c + 1) * 512], pq[:],
                    Act.Copy, scale=-1.0,
                )
            return x2r

        # ---- layer 1 ----
        x2r1 = build_x2r(x0aug16, 3, 32, 64, "x2r1")
        widx1 = _knn_layer(nc, pools, 65, x0aug16, x2r1, st,
                           dump_d=dump_d if dumps else None)
        if dumps:
            nc.sync.dma_start(out=dump_d["dbg_widx"][:], in_=widx1[0][:])
        x1g = const.tile([80, N], f32)   # f32 copy of x1 = EC2 gather source
        nc.vector.memset(x1g[64:80, :], 0.0)
        _edgeconv(nc, pools, 3, 33, 64, 64, x0aug16, x0aug, widx1, ec1_mm1_h,
                  w1b_h, ec1_w2, ec1_b2, ec1_w3, ec1_b3, x1g, 16)
        if dumps:
            nc.sync.dma_start(out=dump_d["dbg_x1g"][:], in_=x1g[0:64, 0:16])
            nc.sync.dma_start(out=dump_d["dbg_x1_full"][:], in_=x1g[0:64, :])

        # ---- layer 2 ----
        x1aug16 = sb.tile([97, N], f16, tag="xaug16", bufs=1, name="x1aug16")
        nc.vector.memset(x1aug16[64:97, :], 0.0)
        nc.vector.memset(x1aug16[64:65, :], 1.0)
        nc.vector.tensor_copy(x1aug16[0:64, :], x1g[0:64, :])
        x2r2 = build_x2r(x1aug16, 64, 64, 96, "x2r2")
        widx2 = _knn_layer(nc, pools, 97, x1aug16, x2r2, st)
        x2f = const.tile([P, 2 * N], f16)
        _edgeconv(nc, pools, 64, 65, P, 256, x1aug16, x1g, widx2, ec2_mm1_h,
                  w2b_h, ec2_w2, ec2_b2, ec2_w3, ec2_b3, x2f, 80)

        # ---- lin0 (feature-major) + segment max ----
        l0w_a = to16(load(lin0_w[0:128, :], [P, 512])[:], [P, 512])
        l0w_b = to16(load(lin0_w[128:256, :], [P, 512])[:], [P, 512])
        l0b = load(lin0_b[:].rearrange("(o x) -> x o", o=4), [P, 4])
        pselr = const.tile([P, 16], dt.int16)
        hselr = const.tile([P, B * SEG_PTS // 16], dt.int16)
        for g in range(8):
            nc.sync.dma_start(out=pselr[g * 16 : (g + 1) * 16, :], in_=psel_d[:])
            nc.sync.dma_start(out=hselr[g * 16 : (g + 1) * 16, :], in_=hsel_d[:])
        pmax = const.tile([P, 4 * B], f32)
        HW = N + CH
        for b_ in range(4):
            hfm = sb.tile([P, HW], f32, tag="hfm", bufs=1)
            nc.vector.memset(hfm[:, N:HW], NEG)
            for c in range(N // 512):
                pq = psum.tile([P, 512], f32, tag="mlp")
                for kk in range(2):
                    l0w = l0w_a if kk == 0 else l0w_b
                    nc.tensor.matmul(
                        pq[:],
                        l0w[:, b_ * P : (b_ + 1) * P],
                        x2f[:, kk * N + c * 512 : kk * N + (c + 1) * 512],
                        start=(kk == 0),
                        stop=(kk == 1),
                    )
                nc.scalar.activation(
                    hfm[:, c * 512 : (c + 1) * 512], pq[:], Act.Relu,
                    bias=l0b[:, b_ : b_ + 1],
                )
            TM = smalls.tile([P, 33], f32, tag="TM")
            nc.vector.memset(TM[:, 32:33], NEG)
            nc.vector.tensor_reduce(
                out=TM[:, 0:32],
                in_=hfm[:, 0:N].rearrange("c (t p) -> c t p", p=P),
                axis=mybir.AxisListType.X,
                op=Alu.max,
            )
            if dumps and b_ == 0:
                nc.sync.dma_start(out=dump_d["dbg_hfm"][:], in_=hfm[:, 0:16])
                nc.sync.dma_start(out=dump_d["dbg_tm"][:], in_=TM[:])
                nc.sync.dma_start(out=dump_d["dbg_hfm0"][:], in_=hfm[:, 0:N])
            gpt = smalls.tile([P, B * 32], f32, tag="gpt")
            nc.gpsimd.ap_gather(
                out_ap=gpt[:].rearrange("c (i one) -> c i one", one=1),
                in_ap=TM[:].rearrange("c (e one) -> c e one", one=1),
                idxs_ap=pselr[:],
                channels=P, num_elems=33, d=1, num_idxs=B * 32,
            )
            gbd = smalls.tile([P, B * SEG_PTS], f32, tag="gbd", bufs=1)
            nc.gpsimd.ap_gather(
                out_ap=gbd[:].rearrange("c (i one) -> c i one", one=1),
                in_ap=hfm[:, 0 : N + 1].rearrange("c (e one) -> c e one", one=1),
                idxs_ap=hselr[:],
                channels=P, num_elems=N + 1, d=1, num_idxs=B * SEG_PTS,
            )
            pa = smalls.tile([P, B], f32, tag="pa")
            nc.vector.tensor_reduce(
                out=pa[:], in_=gpt[:].rearrange("c (s i) -> c s i", i=32),
                axis=mybir.AxisListType.X, op=Alu.max,
            )
            pb = smalls.tile([P, B], f32, tag="pb")
            nc.vector.tensor_reduce(
                out=pb[:], in_=gbd[:].rearrange("c (s i) -> c s i", i=SEG_PTS),
                axis=mybir.AxisListType.X, op=Alu.max,
            )
            nc.vector.tensor_tensor(
                out=pmax[:, b_ * B : (b_ + 1) * B], in0=pa[:], in1=pb[:],
                op=Alu.max,
            )
        if dumps:
            nc.sync.dma_start(out=dump_d["dbg_pmax"][:], in_=pmax[:])
            x2fd = const.tile([P, 16], f32)
            nc.vector.tensor_copy(x2fd[:], x2f[:, 0:16])
            nc.sync.dma_start(out=dump_d["dbg_x2f"][:], in_=x2fd[:])
            nc.sync.dma_start(out=dump_d["dbg_x2_full"][:], in_=x2f[:])
        # ---- AllReduce-max across the 8 cores ----
        if collective:
            ccw = nc.sync.dma_start(out=cc_in[:], in_=pmax[:])
            cc = nc.gpsimd.collective_compute(
                "AllReduce", Alu.max, replica_groups=[list(range(B))],
                ins=[cc_in[:]], outs=[cc_out[:]],
            )
            _add_dep_helper(cc.ins, ccw.ins, sync=True,
                            reason="cc_in write -> allreduce")
            smax = const.tile([P, 4 * B], f32)
            ccr = nc.sync.dma_start(out=smax[:], in_=cc_out[:])
            _add_dep_helper(ccr.ins, cc.ins, sync=True,
                            reason="allreduce -> smax load")
        else:
            smax = pmax
        if dumps:
            nc.sync.dma_start(out=dump_d["dbg_smax"][:], in_=smax[:])

        # ---- head ----
        ones8 = const.tile([1, B], f32)
        nc.vector.memset(ones8[:], 1.0)

        def linear(x_blocks, w_dr, b_dr, kin, kout, relu, nm):
            pq = psum.tile([B, kout], f32, tag="tp")
            nb = (kin + P - 1) // P
            for kk in range(nb):
                kw = min(P, kin - kk * P)
                wt = load(w_dr[kk * P : kk * P + kw, :], [kw, kout], smalls,
                          tag="hw_" + nm, bufs=1)
                nc.tensor.matmul(
                    pq[:], x_blocks[kk][0:kw, 0:B], wt[:], start=(kk == 0),
                    stop=False,
                )
            bt = load(b_dr[:].rearrange("(o x) -> o x", o=1), [1, kout], smalls,
                      tag="hb_" + nm, bufs=1)
            nc.tensor.matmul(pq[:], ones8[:], bt[:], start=False, stop=True)
            o = smalls.tile([B, kout], f32, tag="ho_" + nm)
            nc.scalar.activation(o[:], pq[:], Act.Relu if relu else Act.Copy)
            return o

        def to_blocks(x, kout, nm):
            blocks = []
            for kk in range(kout // P):
                tp = psum.tile([P, B], f32, tag="tp")
                nc.tensor.transpose(
                    tp[:], x[:, kk * P : (kk + 1) * P], identity[0:B, 0:B]
                )
                s = smalls.tile([P, B], f32, tag="ht_" + nm)
                nc.vector.tensor_copy(s[:], tp[:])
                blocks.append(s)
            return blocks

        smax_blocks = [smax[:, b_ * B : (b_ + 1) * B] for b_ in range(4)]
        h1h = linear(smax_blocks, lin1_w, lin1_b, 512, 256, True, "l1")
        h1b = [b_[:] for b_ in to_blocks(h1h[:], 256, "l1")]
        h2h = linear(h1b, lin2_w, lin2_b, 256, 256, True, "l2")
        h2b = [b_[:] for b_ in to_blocks(h2h[:], 256, "l2")]
        h3h = linear(h2b, lin3_w, lin3_b, 256, NCLS, False, "l3")
        # log_softmax
        rmax = smalls.tile([B, 1], f32, tag="rmax")
        nc.vector.tensor_reduce(
            out=rmax[:], in_=h3h[:], axis=mybir.AxisListType.X, op=Alu.max
        )
        shifted = smalls.tile([B, NCLS], f32, tag="shifted")
        nc.vector.tensor_scalar(
            out=shifted[:], in0=h3h[:], scalar1=rmax[:, 0:1], scalar2=None,
            op0=Alu.subtract,
        )
        expacc = smalls.tile([B, 1], f32, tag="expacc")
        expt = smalls.tile([B, NCLS], f32, tag="expt")
        nc.scalar.activation(expt[:], shifted[:], Act.Exp, accum_out=expacc[:])
        lnz = smalls.tile([B, 1], f32, tag="lnz")
        nc.scalar.activation(lnz[:], expacc[:], Act.Ln)
        outt = smalls.tile([B, NCLS], f32, tag="outt")
        nc.vector.tensor_scalar(
            out=outt[:], in0=shifted[:], scalar1=lnz[:, 0:1], scalar2=None,
            op0=Alu.subtract,
        )
        nc.sync.dma_start(out=out_d[:], in_=outt[:])

    nc.finalize()
    return nc


# --------------------------------------------------------------------------
def _host_prep(pos, batch):
    pos = np.asarray(pos, dtype=np.float32)
    batch = np.asarray(batch, dtype=np.int32)
    maps = []
    for c in range(B):
        pb = pos[c * N : (c + 1) * N]
        bb = batch[c * N : (c + 1) * N]
        x0aug = np.zeros((65, N), dtype=np.float32)
        x0aug[0:3] = pb.T
        x0aug[32] = 1.0
        psel = np.full((B, 32), 32, dtype=np.int16)     # 32 -> -inf slot
        hsel = np.full((B, SEG_PTS), N, dtype=np.int16)  # N -> -inf column
        for s in range(B):
            idx = np.nonzero(bb == s)[0]
            if idx.size == 0:
                continue
            t0, t1 = idx[0] // P, idx[-1] // P
            pure, bnd = [], []
            for t in range(t0, t1 + 1):
                lo, hi = t * P, (t + 1) * P
                if idx[0] <= lo and idx[-1] >= hi - 1:
                    pure.append(t)
                else:
                    bnd.extend(range(max(lo, int(idx[0])), min(hi, int(idx[-1]) + 1)))
            psel[s, : len(pure)] = pure
            assert len(bnd) <= SEG_PTS
            hsel[s, : len(bnd)] = bnd
        maps.append({
            "x0aug": x0aug,
            "psel": _wrap16(psel.reshape(-1), 16),
            "hsel": _wrap16(hsel.reshape(-1), B * SEG_PTS // 16),
        })
    return maps


_WNAMES = ["m1w1", "m1b1", "m1w2", "m1b2", "m1w3", "m1b3",
           "m2w1", "m2b1", "m2w2", "m2b2", "m2w3", "m2b3",
           "lin0_w", "lin0_b", "lin1_w", "lin1_b", "lin2_w", "lin2_b",
           "lin3_w", "lin3_b"]
_CACHE = {}


def kernel(**inputs):
    from concourse.bass_utils import run_bass_kernel_spmd

    if "nc" not in _CACHE:
        _CACHE["nc"] = build()
    maps = _host_prep(inputs["pos"], inputs["batch"])
    for m in maps:
        for w in _WNAMES:
            m[w] = np.ascontiguousarray(np.asarray(inputs[w], dtype=np.float32))
    res = run_bass_kernel_spmd(_CACHE["nc"], maps, core_ids=list(range(B)))
    return np.asarray(res.results[0]["out"], dtype=np.float32)


# revision 66
# speedup vs baseline: 1.0477x; 1.0477x over previous
"""DGCNN forward (2x dynamic-kNN EdgeConv + classifier) on 8 Trainium2 cores.

Data-parallel over the B=8 point clouds: core b handles cloud b (4096 points),
fully fused on-chip:

  kNN   - augmented f16 matmul gives negdist = 2*x_i.x_j - |x_j|^2 - |x_i|^2
          straight in PSUM (1 PE cycle/row vs 4 for f32); the self column is
          pre-killed with a predicated diagonal write; one DVE pass packs the
          column index j into the low 12 mantissa bits (negdist quantized to
          ~2^-11 relative); a sub-chunk(8)-max reduce gives 512 packed
          sub-chunk maxima per point and three max8/match_replace rounds
          select the top-24; the top-20 sub-chunk maxima ARE the neighbor
          list (approximate kNN: a true neighbor is only replaced when two of
          the top-20 share an 8-wide index sub-chunk, which the EdgeConv max
          aggregation washes out). Everything stays in SBUF - no DRAM
          round-trip, no indirect DMA.
  EConv - neighbor features move SBUF->SBUF with ap_gather driven by a
          wrapped index list built with two PE transposes; the MLP runs
          feature-major in f16 (stationary weights, 1 cycle/row); max over
          the 20 neighbors is fused into the PSUM evacuation of the last
          layer.
  Head  - lin0 feature-major f16; segment-max uses host-prepared index lists
          (from the sorted batch labels) gathered against -inf-augmented
          tables; a 16KB AllReduce-max merges the per-core [8,512] partials;
          each core runs the tiny classifier + log_softmax in f32.
"""

import contextlib

import numpy as np

import bass_rust
import concourse.bass as bass
import concourse.bacc as bacc
import concourse.mybir as mybir
from concourse import masks
from concourse.bass import _add_dep_helper
from concourse.tile import TileContext
from concourse.vector_clock import ScopedClock

dt = mybir.dt
Alu = mybir.AluOpType
Act = mybir.ActivationFunctionType

B, N, KNN, NCLS = 8, 4096, 20, 40
P = 128
NT = N // P            # 32 point tiles per core
CH = 8                 # sub-chunk size for the maxima hierarchy
NCH = N // CH          # 512 sub-chunks per row
NC3 = 24               # sub-chunk maxima kept (3 max8 rounds)
KPAD = 32              # padded K for the wrapped gather list
NEG = -1.0e30
MASK_HI = -4096        # 0xFFFFF000 as signed int32
SEG_PTS = 256          # boundary-point slots per segment
Q = 1024               # dist quarter width
NQ = N // Q


# --------------------------------------------------------------------------
# This walrus build rejects Drain instructions carrying >1 sync wait; split
# the TileContext tail-drain waits across single-wait nops.
def _patched_drain_and_barrier(self, tick_clock, wait_clock):
    nc = self.nc
    probe = nc.sync.nop(nofuse=True)
    wait_clock.add_sem_waits(probe.ins, ScopedClock({None: tick_clock.global_clock}))
    si = probe.ins.sync_info
    waits = list(si.on_wait) if si is not None else []
    if len(waits) > 1:
        probe.ins.sync_info = bass_rust.SyncInfo(
            on_wait=[waits[0]], on_update=list(si.on_update)
        )
        for w in waits[1:]:
            extra = nc.sync.nop(nofuse=True)
            extra.ins.sync_info = bass_rust.SyncInfo(on_wait=[w], on_update=[])
    nc.sync.drain()
    nc.all_engine_barrier()
    assert self.sems is not None
    popped = nc._tile_sem_poison_stack.pop()
    assert popped is self._sem_poison
    nc.clear_and_free_semaphores(list(self.sems.allocated().values()))
    nc.all_engine_barrier()


TileContext._drain_and_barrier = _patched_drain_and_barrier


def _wrap16(lst, cols):
    """[i % 16, i // 16] wrapped layout used by the gpsimd gather ops."""
    a = np.asarray(lst, dtype=np.int16)
    assert a.size == 16 * cols, (a.size, cols)
    return a.reshape(cols, 16).T.copy()


# --------------------------------------------------------------------------
def _knn_layer(nc, pools, KA, x_aug, x2r, st, dump_d=None):
    """One kNN layer (fully in SBUF); returns per-tile wrapped gather lists."""
    sb, psum, smalls = pools["sb"], pools["psum"], pools["smalls"]
    widx_tiles = []
    for t in range(NT):
        lhsT = x_aug[0:KA, t * P : (t + 1) * P]
        cmax = smalls.tile([P, NCH], dt.float32, tag="cmax")
        for q in range(NQ):
            pq = psum.tile([P, Q], dt.float32, tag="dist", bufs=1)
            for h in range(Q // 512):
                j0 = q * Q + h * 512
                nc.tensor.matmul(
                    pq[:, h * 512 : (h + 1) * 512],
                    lhsT,
                    x2r[0:KA, j0 : j0 + 512],
                    start=True,
                    stop=True,
                )
            if q == (t * P) // Q:
                # negdist(i,i) == 0 is always the row max; kill it in PSUM
                off = (t * P) % Q
                nc.vector.copy_predicated(
                    pq[:, off : off + P],
                    st["identity"][:].bitcast(dt.uint32),
                    st["negbig"][:],
                )
            packed = sb.tile([P, Q], dt.float32, tag="packed")
            # packed = (negdist & 0xFFFFF000) | j
            sttp = nc.vector.scalar_tensor_tensor(
                out=packed[:].bitcast(dt.int32),
                in0=pq[:].bitcast(dt.int32),
                scalar=MASK_HI,
                in1=st["iota_j"][:, q * Q : (q + 1) * Q],
                op0=Alu.bitwise_and,
                op1=Alu.bitwise_or,
            )
            # walrus wants an integer ImmVal for bitvec ops
            _il = sttp.ins.ins
            _il[1] = mybir.ImmediateValue(dtype=dt.int32, value=MASK_HI)
            sttp.ins.ins = _il
            nc.vector.tensor_reduce(
                out=cmax[:, q * (Q // CH) : (q + 1) * (Q // CH)],
                in_=packed[:].rearrange("p (c s) -> p c s", s=CH),
                axis=mybir.AxisListType.X,
                op=Alu.max,
            )
        # ---- top-24 sub-chunk maxima; ids live in the low 12 bits ----
        if t == 0 and dump_d is not None:
            nc.sync.dma_start(out=dump_d["dbg_cmax"][:], in_=cmax[:])
        m8 = smalls.tile([P, NC3], dt.float32, tag="m8")
        cwork = smalls.tile([P, NCH], dt.float32, tag="cwork")
        src = cmax
        for r in range(3):
            nc.vector.max(out=m8[:, r * 8 : (r + 1) * 8], in_=src[:])
            if r < 2:
                nc.vector.match_replace(
                    out=cwork[:],
                    in_to_replace=m8[:, r * 8 : (r + 1) * 8],
                    in_values=src[:],
                    imm_value=NEG,
                )
                src = cwork
        if t == 0 and dump_d is not None:
            nc.sync.dma_start(out=dump_d["dbg_m8"][:], in_=m8[:])
        gidx = smalls.tile([P, KNN], dt.int32, tag="gidx")
        nc.vector.tensor_scalar(
            out=gidx[:], in0=m8[:, 0:KNN].bitcast(dt.int32), scalar1=0xFFF,
            scalar2=None, op0=Alu.bitwise_and,
        )
        # ---- wrapped gather list via two PE transposes ----
        gf = smalls.tile([P, KPAD], dt.float32, tag="gf")
        nc.vector.tensor_copy(gf[:, 0:KNN], gidx[:])
        nc.vector.tensor_copy(
            gf[:, KNN:KPAD], gidx[:, 0:1].to_broadcast([P, KPAD - KNN])
        )
        w32 = smalls.tile([16, 2 * P], dt.float32, tag="w32")
        w32v = w32[:].rearrange("r (c two) -> r c two", two=2)
        for half in range(2):
            tp = psum.tile([16, P], dt.float32, tag="tp", bufs=1)
            nc.tensor.transpose(
                tp[:], gf[:, half * 16 : (half + 1) * 16], st["identity"][:]
            )
            nc.vector.tensor_copy(w32v[:, :, half], tp[:])
        widx = smalls.tile([16, 2 * P], dt.int16, tag="widx", bufs=4)
        nc.vector.tensor_copy(widx[:], w32[:])
        widx_tiles.append(widx)
    return widx_tiles


# --------------------------------------------------------------------------
def _edgeconv(nc, pools, D, KA, DMID, DOUT, x_aug, x_gsrc, widx_tiles, wmm1,
              wmm2, w2t, b2t, w3t, b3t, x_out, gtab_rows):
    """One EdgeConv; pooled relu output written to x_out (feature-major,
    [P, (DOUT//P or 1)*N] layout, block b at columns [b*N, (b+1)*N)).

    x_aug is the f16 feature tensor (rhs_i / matmul side); x_gsrc is an f32
    copy of the feature rows used as the ap_gather source (the gpsimd gather
    needs 4-byte elements); gathered rows are converted to f16 per tile."""
    sb, psum, smalls = pools["sb"], pools["psum"], pools["smalls"]
    chunks = [(0, 25), (25, 25), (50, 25), (75, 25), (100, 25), (125, 3)]
    NB3 = max(1, DOUT // P)
    for t in range(NT):
        widx = widx_tiles[t]
        gath = sb.tile([gtab_rows, P * KPAD], dt.float32, tag="gath", bufs=1)
        if gtab_rows > 16:
            wrep = smalls.tile([gtab_rows, 2 * P], dt.int16, tag="wrep")
            for g in range(gtab_rows // 16):
                nc.sync.dma_start(out=wrep[g * 16 : (g + 1) * 16, :], in_=widx[:])
            idxs = wrep
        else:
            idxs = widx
        nc.gpsimd.ap_gather(
            out_ap=gath[:].rearrange("c (i one) -> c i one", one=1),
            in_ap=x_gsrc[0:gtab_rows, :].rearrange("c (e one) -> c e one", one=1),
            idxs_ap=idxs[:],
            channels=gtab_rows,
            num_elems=N,
            d=1,
            num_idxs=P * KPAD,
        )
        gath16 = sb.tile([D, P * KPAD], dt.float16, tag="gath16")
        nc.vector.tensor_copy(gath16[:], gath[0:D, :])
        gview = gath16[:].rearrange("c (p k) -> c p k", k=KPAD)
        for (p0, pn) in chunks:
            ns = pn * KNN
            # h1 = relu(x_i @ (W1a-W1b) + b1 + x_j @ W1b)
            ph1 = psum.tile([DMID, 512], dt.float32, tag="mlp1")
            rhs_i = (
                x_aug[0:KA, t * P + p0 : t * P + p0 + pn]
                .rearrange("c (p one) -> c p one", one=1)
                .to_broadcast([KA, pn, KNN])
            )
            nc.tensor.matmul(ph1[:, 0:ns], wmm1[:], rhs_i, start=True, stop=False)
            nc.tensor.matmul(
                ph1[:, 0:ns], wmm2[:], gview[0:D, p0 : p0 + pn, 0:KNN],
                start=False, stop=True,
            )
            h1 = sb.tile([DMID, 512], dt.float16, tag="h1")
            nc.scalar.activation(h1[:, 0:ns], ph1[:, 0:ns], Act.Relu)
            # h2 = relu(h1 @ W2 + b2)
            ph2 = psum.tile([DMID, 512], dt.float32, tag="mlp2")
            nc.tensor.matmul(ph2[:, 0:ns], w2t[:], h1[:, 0:ns], start=True, stop=True)
            h2 = sb.tile([DMID, 512], dt.float16, tag="h2")
            nc.scalar.activation(
                h2[:, 0:ns], ph2[:, 0:ns], Act.Relu, bias=b2t[:, 0:1]
            )
            # h3 = h2 @ W3 ; max over k ; relu(. + b3)
            for b3 in range(NB3):
                mw = min(P, DOUT)
                ph3 = psum.tile([P, 512], dt.float32, tag="mlp3", bufs=1)
                nc.tensor.matmul(
                    ph3[0:mw, 0:ns], w3t[:, b3 * P : b3 * P + mw], h2[:, 0:ns],
                    start=True, stop=True,
                )
                pooled = smalls.tile([P, 32], dt.float32, tag="pooled")
                nc.vector.tensor_reduce(
                    out=pooled[0:mw, 0:pn],
                    in_=ph3[0:mw, 0:ns].rearrange("c (p k) -> c p k", k=KNN),
                    axis=mybir.AxisListType.X,
                    op=Alu.max,
                )
                nc.scalar.activation(
                    x_out[0:mw, b3 * N + t * P + p0 : b3 * N + t * P + p0 + pn],
                    pooled[0:mw, 0:pn],
                    Act.Relu,
                    bias=b3t[0:mw, b3 : b3 + 1],
                )


# --------------------------------------------------------------------------
def build(collective=True, debug=False, dumps=False):
    nc = bacc.Bacc(
        "TRN2", target_bir_lowering=False, debug=debug,
        num_devices=B if collective else 1,
    )
    f32 = dt.float32
    dump_d = {}
    if dumps:
        for nm, shp, dtp in [
            ("dbg_x0", [65, 16], f32), ("dbg_widx", [16, 256], dt.int16),
            ("dbg_x1g", [64, 16], f32), ("dbg_pmax", [P, 4 * B], f32),
            ("dbg_smax", [P, 4 * B], f32), ("dbg_x2f", [P, 16], f32),
            ("dbg_iota", [1, 64], dt.int32),
            ("dbg_cmax", [P, NCH], f32), ("dbg_m8", [P, NC3], f32),
            ("dbg_hfm", [P, 16], f32),
            ("dbg_tm", [P, 33], f32), ("dbg_x1_full", [64, N], f32),
            ("dbg_x2_full", [P, 2 * N], dt.float16),
            ("dbg_hfm0", [P, N], f32),
        ]:
            dump_d[nm] = nc.dram_tensor(nm, shp, dtp, kind="ExternalOutput")

    def din(name, shape, dtype=f32):
        return nc.dram_tensor(name, shape, dtype, kind="ExternalInput")

    x0aug_d = din("x0aug", [65, N])
    psel_d = din("psel", [16, 16], dt.int16)
    hsel_d = din("hsel", [16, B * SEG_PTS // 16], dt.int16)
    w_m1 = din("m1w1", [6, 64]); b_m1 = din("m1b1", [64])
    w_m12 = din("m1w2", [64, 64]); b_m12 = din("m1b2", [64])
    w_m13 = din("m1w3", [64, 64]); b_m13 = din("m1b3", [64])
    w_m2 = din("m2w1", [P, P]); b_m2 = din("m2b1", [P])
    w_m22 = din("m2w2", [P, P]); b_m22 = din("m2b2", [P])
    w_m23 = din("m2w3", [P, 256]); b_m23 = din("m2b3", [256])
    lin0_w = din("lin0_w", [256, 512]); lin0_b = din("lin0_b", [512])
    lin1_w = din("lin1_w", [512, 256]); lin1_b = din("lin1_b", [256])
    lin2_w = din("lin2_w", [256, 256]); lin2_b = din("lin2_b", [256])
    lin3_w = din("lin3_w", [256, NCLS]); lin3_b = din("lin3_b", [NCLS])
    out_d = nc.dram_tensor("out", [B, NCLS], f32, kind="ExternalOutput")

    cc_in = nc.dram_tensor("cc_in", [P, 4 * B], f32)
    cc_out = nc.dram_tensor("cc_out", [P, 4 * B], f32, addr_space="Shared")

    with TileContext(nc) as tc, contextlib.ExitStack() as ctx:
        const = ctx.enter_context(tc.tile_pool(name="const", bufs=1))
        sb = ctx.enter_context(tc.tile_pool(name="sb", bufs=2))
        smalls = ctx.enter_context(tc.tile_pool(name="smalls", bufs=2))
        psum = ctx.enter_context(tc.tile_pool(name="psum", bufs=2, space="PSUM"))
        pools = {"sb": sb, "psum": psum, "smalls": smalls}

        f16 = dt.float16

        # ---- statics ----
        identity = const.tile([P, P], f32)
        masks.make_identity(nc, identity[:])
        iota_j = const.tile([P, N], dt.int32)
        nc.gpsimd.iota(iota_j[:], pattern=[[1, N]], base=0, channel_multiplier=0)
        if dumps:
            nc.sync.dma_start(out=dump_d["dbg_iota"][:], in_=iota_j[0:1, 0:64])
        negbig = const.tile([P, P], f32)
        nc.vector.memset(negbig[:], NEG)
        st = {"identity": identity, "iota_j": iota_j, "negbig": negbig}

        # ---- inputs / weights ----
        x0aug = const.tile([65, N], f32)
        nc.sync.dma_start(out=x0aug[:], in_=x0aug_d[:])
        x0aug16 = sb.tile([97, N], f16, tag="xaug16", bufs=1, name="x0aug16")
        nc.vector.tensor_copy(x0aug16[0:65, :], x0aug[:])
        if dumps:
            nc.sync.dma_start(out=dump_d["dbg_x0"][:], in_=x0aug[:, 0:16])

        _ldn = [0]

        def load(dr_ap, shape, pool=const, tag=None, bufs=None):
            if tag is None:
                _ldn[0] += 1
                tag = f"ld{_ldn[0]}"
            t_ = pool.tile(shape, f32, tag=tag, name=tag, bufs=bufs)
            nc.sync.dma_start(out=t_[:], in_=dr_ap)
            return t_

        def to16(src, shape):
            _ldn[0] += 1
            t_ = const.tile(shape, f16, tag=f"w16_{_ldn[0]}")
            nc.vector.tensor_copy(t_[:], src)
            return t_

        w1a = load(w_m1[0:3, :], [3, 64])
        w1b = load(w_m1[3:6, :], [3, 64])
        ec1_mm1 = const.tile([33, 64], f32)
        nc.vector.memset(ec1_mm1[:], 0.0)
        nc.vector.tensor_sub(ec1_mm1[0:3, :], w1a[:], w1b[:])
        nc.sync.dma_start(
            out=ec1_mm1[32:33, :], in_=b_m1[:].rearrange("(o x) -> o x", o=1)
        )
        ec1_mm1_h = to16(ec1_mm1[:], [33, 64])
        w1b_h = to16(w1b[:], [3, 64])
        ec1_w2 = to16(load(w_m12[:], [64, 64])[:], [64, 64])
        ec1_b2 = load(b_m12[:].rearrange("(x o) -> x o", o=1), [64, 1])
        ec1_w3 = to16(load(w_m13[:], [64, 64])[:], [64, 64])
        ec1_b3 = load(b_m13[:].rearrange("(x o) -> x o", o=1), [64, 1])

        w2a = load(w_m2[0:64, :], [64, P])
        w2b = load(w_m2[64:128, :], [64, P])
        ec2_mm1 = const.tile([65, P], f32)
        nc.vector.tensor_sub(ec2_mm1[0:64, :], w2a[:], w2b[:])
        nc.sync.dma_start(
            out=ec2_mm1[64:65, :], in_=b_m2[:].rearrange("(o x) -> o x", o=1)
        )
        ec2_mm1_h = to16(ec2_mm1[:], [65, P])
        w2b_h = to16(w2b[:], [64, P])
        ec2_w2 = to16(load(w_m22[:], [P, P])[:], [P, P])
        ec2_b2 = load(b_m22[:].rearrange("(x o) -> x o", o=1), [P, 1])
        ec2_w3 = to16(load(w_m23[:], [P, 256])[:], [P, 256])
        ec2_b3 = load(b_m23[:].rearrange("(o x) -> x o", o=2), [P, 2])

        # ---- x2r (dist rhs) builder ----
        def build_x2r(x_aug_t, D, lane1, lane2, tag):
            # negdist = -dist^2: x_aug has ones @ lane1, -|x|^2 @ lane2;
            # x2r has [2x ; -|x|^2 @ lane1 ; ones @ lane2]. All lane starts
            # are 32-aligned. KA = lane2 + 1.
            KA = lane2 + 1
            x2r = sb.tile([KA, N], f16, tag="x2r", bufs=1, name=tag)
            nc.vector.memset(x2r[:], 0.0)
            nc.vector.memset(x2r[lane2 : lane2 + 1, :], 1.0)
            nc.vector.tensor_scalar_mul(x2r[0:D, :], x_aug_t[0:D, :], 2.0)
            xsq = sb.tile([D, N], f16, tag="xsq", bufs=1)
            nc.vector.tensor_mul(xsq[:], x_aug_t[0:D, :], x_aug_t[0:D, :])
            ones_l = const.tile([D, 1], f16, tag=tag + "_ones")
            nc.vector.memset(ones_l[:], 1.0)
            for c in range(N // 512):
                pq = psum.tile([1, 512], f32, tag="tp", bufs=1)
                nc.tensor.matmul(
                    pq[:], ones_l[:], xsq[:, c * 512 : (c + 1) * 512],
                    start=True, stop=True,
                )
                nc.scalar.activation(
                    x2r[lane1 : lane1 + 1, c * 512 : (c + 1) * 512], pq[:],
                    Act.Copy, scale=-1.0,
                )
                nc.scalar.activation(
                    x_aug_t[lane2 : lane2 + 1, c * 512 : (c + 1) * 512], pq[:],
                    Act.Copy, scale=-1.0,
                )
            return x2r

        # ---- layer 1 ----
        x2r1 = build_x2r(x0aug16, 3, 32, 64, "x2r1")
        widx1 = _knn_layer(nc, pools, 65, x0aug16, x2r1, st,
                           dump_d=dump_d if dumps else None)
        if dumps:
            nc.sync.dma_start(out=dump_d["dbg_widx"][:], in_=widx1[0][:])
        x1g = const.tile([80, N], f32)   # f32 copy of x1 = EC2 gather source
        nc.vector.memset(x1g[64:80, :], 0.0)
        _edgeconv(nc, pools, 3, 33, 64, 64, x0aug16, x0aug, widx1, ec1_mm1_h,
                  w1b_h, ec1_w2, ec1_b2, ec1_w3, ec1_b3, x1g, 16)
        if dumps:
            nc.sync.dma_start(out=dump_d["dbg_x1g"][:], in_=x1g[0:64, 0:16])
            nc.sync.dma_start(out=dump_d["dbg_x1_full"][:], in_=x1g[0:64, :])

        # ---- layer 2 ----
        x1aug16 = sb.tile([97, N], f16, tag="xaug16", bufs=1, name="x1aug16")
        nc.vector.memset(x1aug16[64:97, :], 0.0)
        nc.vector.memset(x1aug16[64:65, :], 1.0)
        nc.vector.tensor_copy(x1aug16[0:64, :], x1g[0:64, :])
        x2r2 = build_x2r(x1aug16, 64, 64, 96, "x2r2")
        widx2 = _knn_layer(nc, pools, 97, x1aug16, x2r2, st)
        x2f = const.tile([P, 2 * N], f16)
        _edgeconv(nc, pools, 64, 65, P, 256, x1aug16, x1g, widx2, ec2_mm1_h,
                  w2b_h, ec2_w2, ec2_b2, ec2_w3, ec2_b3, x2f, 80)

        # ---- lin0 (feature-major) + segment max ----
        l0w_a = to16(load(lin0_w[0:128, :], [P, 512])[:], [P, 512])
        l0w_b = to16(load(lin0_w[128:256, :], [P, 512])[:], [P, 512])
        l0b = load(lin0_b[:].rearrange("(o x) -> x o", o=4), [P, 4])
        pselr = const.tile([P, 16], dt.int16)
        hselr = const.tile([P, B * SEG_PTS // 16], dt.int16)
        for g in range(8):
            nc.sync.dma_start(out=pselr[g * 16 : (g + 1) * 16, :], in_=psel_d[:])
            nc.sync.dma_start(out=hselr[g * 16 : (g + 1) * 16, :], in_=hsel_d[:])
        pmax = const.tile([P, 4 * B], f32)
        HW = N + CH
        for b_ in range(4):
            hfm = sb.tile([P, HW], f32, tag="hfm", bufs=1)
            nc.vector.memset(hfm[:, N:HW], NEG)
            for c in range(N // 512):
                pq = psum.tile([P, 512], f32, tag="mlp1")
                for kk in range(2):
                    l0w = l0w_a if kk == 0 else l0w_b
                    nc.tensor.matmul(
                        pq[:],
                        l0w[:, b_ * P : (b_ + 1) * P],
                        x2f[:, kk * N + c * 512 : kk * N + (c + 1) * 512],
                        start=(kk == 0),
                        stop=(kk == 1),
                    )
                nc.scalar.activation(
                    hfm[:, c * 512 : (c + 1) * 512], pq[:], Act.Relu,
                    bias=l0b[:, b_ : b_ + 1],
                )
            TM = smalls.tile([P, 33], f32, tag="TM")
            nc.vector.memset(TM[:, 32:33], NEG)
            nc.vector.tensor_reduce(
                out=TM[:, 0:32],
                in_=hfm[:, 0:N].rearrange("c (t p) -> c t p", p=P),
                axis=mybir.AxisListType.X,
                op=Alu.max,
            )
            if dumps and b_ == 0:
                nc.sync.dma_start(out=dump_d["dbg_hfm"][:], in_=hfm[:, 0:16])
                nc.sync.dma_start(out=dump_d["dbg_tm"][:], in_=TM[:])
                nc.sync.dma_start(out=dump_d["dbg_hfm0"][:], in_=hfm[:, 0:N])
            gpt = smalls.tile([P, B * 32], f32, tag="gpt")
            nc.gpsimd.ap_gather(
                out_ap=gpt[:].rearrange("c (i one) -> c i one", one=1),
                in_ap=TM[:].rearrange("c (e one) -> c e one", one=1),
                idxs_ap=pselr[:],
                channels=P, num_elems=33, d=1, num_idxs=B * 32,
            )
            gbd = smalls.tile([P, B * SEG_PTS], f32, tag="gbd", bufs=1)
            nc.gpsimd.ap_gather(
                out_ap=gbd[:].rearrange("c (i one) -> c i one", one=1),
                in_ap=hfm[:, 0 : N + 1].rearrange("c (e one) -> c e one", one=1),
                idxs_ap=hselr[:],
                channels=P, num_elems=N + 1, d=1, num_idxs=B * SEG_PTS,
            )
            pa = smalls.tile([P, B], f32, tag="pa")
            nc.vector.tensor_reduce(
                out=pa[:], in_=gpt[:].rearrange("c (s i) -> c s i", i=32),
                axis=mybir.AxisListType.X, op=Alu.max,
            )
            pb = smalls.tile([P, B], f32, tag="pb")
            nc.vector.tensor_reduce(
                out=pb[:], in_=gbd[:].rearrange("c (s i) -> c s i", i=SEG_PTS),
                axis=mybir.AxisListType.X, op=Alu.max,
            )
            nc.vector.tensor_tensor(
                out=pmax[:, b_ * B : (b_ + 1) * B], in0=pa[:], in1=pb[:],
                op=Alu.max,
            )
        if dumps:
            nc.sync.dma_start(out=dump_d["dbg_pmax"][:], in_=pmax[:])
            x2fd = const.tile([P, 16], f32)
            nc.vector.tensor_copy(x2fd[:], x2f[:, 0:16])
            nc.sync.dma_start(out=dump_d["dbg_x2f"][:], in_=x2fd[:])
            nc.sync.dma_start(out=dump_d["dbg_x2_full"][:], in_=x2f[:])
        # ---- AllReduce-max across the 8 cores ----
        if collective:
            ccw = nc.sync.dma_start(out=cc_in[:], in_=pmax[:])
            cc = nc.gpsimd.collective_compute(
                "AllReduce", Alu.max, replica_groups=[list(range(B))],
                ins=[cc_in[:]], outs=[cc_out[:]],
            )
            _add_dep_helper(cc.ins, ccw.ins, sync=True,
                            reason="cc_in write -> allreduce")
            smax = const.tile([P, 4 * B], f32)
            ccr = nc.sync.dma_start(out=smax[:], in_=cc_out[:])
            _add_dep_helper(ccr.ins, cc.ins, sync=True,
                            reason="allreduce -> smax load")
        else:
            smax = pmax
        if dumps:
            nc.sync.dma_start(out=dump_d["dbg_smax"][:], in_=smax[:])

        # ---- head ----
        ones8 = const.tile([1, B], f32)
        nc.vector.memset(ones8[:], 1.0)

        def linear(x_blocks, w_dr, b_dr, kin, kout, relu, nm):
            pq = psum.tile([B, kout], f32, tag="tp", bufs=1)
            nb = (kin + P - 1) // P
            for kk in range(nb):
                kw = min(P, kin - kk * P)
                wt = load(w_dr[kk * P : kk * P + kw, :], [kw, kout], smalls,
                          tag="hw_" + nm, bufs=1)
                nc.tensor.matmul(
                    pq[:], x_blocks[kk][0:kw, 0:B], wt[:], start=(kk == 0),
                    stop=False,
                )
            bt = load(b_dr[:].rearrange("(o x) -> o x", o=1), [1, kout], smalls,
                      tag="hb_" + nm, bufs=1)
            nc.tensor.matmul(pq[:], ones8[:], bt[:], start=False, stop=True)
            o = smalls.tile([B, kout], f32, tag="ho_" + nm)
            nc.scalar.activation(o[:], pq[:], Act.Relu if relu else Act.Copy)
            return o

        def to_blocks(x, kout, nm):
            blocks = []
            for kk in range(kout // P):
                tp = psum.tile([P, B], f32, tag="tp", bufs=1)
                nc.tensor.transpose(
                    tp[:], x[:, kk * P : (kk + 1) * P], identity[0:B, 0:B]
                )
                s = smalls.tile([P, B], f32, tag="ht_" + nm)
                nc.vector.tensor_copy(s[:], tp[:])
                blocks.append(s)
            return blocks

        smax_blocks = [smax[:, b_ * B : (b_ + 1) * B] for b_ in range(4)]
        h1h = linear(smax_blocks, lin1_w, lin1_b, 512, 256, True, "l1")
        h1b = [b_[:] for b_ in to_blocks(h1h[:], 256, "l1")]
        h2h = linear(h1b, lin2_w, lin2_b, 256, 256, True, "l2")
        h2b = [b_[:] for b_ in to_blocks(h2h[:], 256, "l2")]
        h3h = linear(h2b, lin3_w, lin3_b, 256, NCLS, False, "l3")
        # log_softmax
        rmax = smalls.tile([B, 1], f32, tag="rmax")
        nc.vector.tensor_reduce(
            out=rmax[:], in_=h3h[:], axis=mybir.AxisListType.X, op=Alu.max
        )
        shifted = smalls.tile([B, NCLS], f32, tag="shifted")
        nc.vector.tensor_scalar(
            out=shifted[:], in0=h3h[:], scalar1=rmax[:, 0:1], scalar2=None,
            op0=Alu.subtract,
        )
        expacc = smalls.tile([B, 1], f32, tag="expacc")
        expt = smalls.tile([B, NCLS], f32, tag="expt")
        nc.scalar.activation(expt[:], shifted[:], Act.Exp, accum_out=expacc[:])
        lnz = smalls.tile([B, 1], f32, tag="lnz")
        nc.scalar.activation(lnz[:], expacc[:], Act.Ln)
        outt = smalls.tile([B, NCLS], f32, tag="outt")
        nc.vector.tensor_scalar(
            out=outt[:], in0=shifted[:], scalar1=lnz[:, 0:1], scalar2=None,
            op0=Alu.subtract,
        )
        nc.sync.dma_start(out=out_d[:], in_=outt[:])

    nc.finalize()
    return nc


# --------------------------------------------------------------------------
def _host_prep(pos, batch):
    pos = np.asarray(pos, dtype=np.float32)
    batch = np.asarray(batch, dtype=np.int32)
    maps = []
    for c in range(B):
        pb = pos[c * N : (c + 1) * N]
        bb = batch[c * N : (c + 1) * N]
        x0aug = np.zeros((65, N), dtype=np.float32)
        x0aug[0:3] = pb.T
        x0aug[32] = 1.0
        psel = np.full((B, 32), 32, dtype=np.int16)     # 32 -> -inf slot
        hsel = np.full((B, SEG_PTS), N, dtype=np.int16)  # N -> -inf column
        for s in range(B):
            idx = np.nonzero(bb == s)[0]
            if idx.size == 0:
                continue
            t0, t1 = idx[0] // P, idx[-1] // P
            pure, bnd = [], []
            for t in range(t0, t1 + 1):
                lo, hi = t * P, (t + 1) * P
                if idx[0] <= lo and idx[-1] >= hi - 1:
                    pure.append(t)
                else:
                    bnd.extend(range(max(lo, int(idx[0])), min(hi, int(idx[-1]) + 1)))
            psel[s, : len(pure)] = pure
            assert len(bnd) <= SEG_PTS
            hsel[s, : len(bnd)] = bnd
        maps.append({
            "x0aug": x0aug,
            "psel": _wrap16(psel.reshape(-1), 16),
            "hsel": _wrap16(hsel.reshape(-1), B * SEG_PTS // 16),
        })
    return maps


_WNAMES = ["m1w1", "m1b1", "m1w2", "m1b2", "m1w3", "m1b3",
           "m2w1", "m2b1", "m2w2", "m2b2", "m2w3", "m2b3",
           "lin0_w", "lin0_b", "lin1_w", "lin1_b", "lin2_w", "lin2_b",
           "lin3_w", "lin3_b"]
_CACHE = {}


def kernel(**inputs):
    from concourse.bass_utils import run_bass_kernel_spmd

    if "nc" not in _CACHE:
        _CACHE["nc"] = build()
    maps = _host_prep(inputs["pos"], inputs["batch"])
    for m in maps:
        for w in _WNAMES:
            m[w] = np.ascontiguousarray(np.asarray(inputs[w], dtype=np.float32))
    res = run_bass_kernel_spmd(_CACHE["nc"], maps, core_ids=list(range(B)))
    return np.asarray(res.results[0]["out"], dtype=np.float32)
